# revision 2
# baseline (speedup 1.0000x reference)
"""Trainium2 Bass kernel for nn_Attention_78847009620267.

Reference computation (per batch b):
    t1 = fv[b] @ W_f + (b_f + hidden[b] @ W_h + b_h)     # [L, I]
    e  = t1 @ W_a + b_a                                  # [L, D]
    alpha = softmax(e, axis=L)                           # [L, D]
    z  = sum_l alpha * fv[b]                             # [D]
returns (z [B, D], alpha [B, L, D])

Strategy: data-parallel over batch B=64 across 8 NeuronCores (8 batches per
core), weights replicated.  The whole device kernel runs in the transposed
[D, L] domain so that softmax reductions, bias adds, and normalization are
all native per-partition operations; the host pre-transposes fv into
[b, p, c, l] layout (d = c*128 + p) and post-transposes alpha back.  All
matmuls run in bf16 (fp32 accumulation in PSUM); exp's softmax denominator
comes for free from the ScalarE activation accum_out.

No collectives are needed: each core's outputs depend only on its shard.
"""

import os
import sys

import numpy as np

sys.path.insert(0, "/opt/trn_rl_repo")

B, L, D, I = 64, 196, 2048, 512
NCORES = 8
BSH = B // NCORES          # batches per core = 8
DC = D // 128              # 16 d-chunks
IC = I // 128              # 4 i-chunks
NPAIR = BSH // 2           # process 2 batches per pass (N = 2*L = 392)

_CACHE = {}


def _build_nc():
    """Build the per-core Bass graph (same graph for all 8 cores)."""
    import concourse.bass as bass  # noqa: F401
    import concourse.mybir as mybir
    import concourse.tile as tile
    from concourse import bacc

    dt = mybir.dt
    AF = mybir.ActivationFunctionType
    ALU = mybir.AluOpType

    nc = bacc.Bacc("TRN2", target_bir_lowering=False, debug=False)

    # ---- I/O declarations (per-core shard layouts, see host code below) ----
    # xt[b, p, c, l] = fv[b, l, c*128 + p]  (f32; cast to bf16 during DMA)
    xt = nc.dram_tensor("xt", [BSH, 128, DC, L], dt.float32, kind="ExternalInput")
    # wf[p, k, i] = W_f[k*128 + p, i]  (bf16)
    wf = nc.dram_tensor("wf", [128, DC, I], dt.bfloat16, kind="ExternalInput")
    # wa[p, k, d] = W_a[k*128 + p, d]  (bf16)
    wa = nc.dram_tensor("wa", [128, IC, D], dt.bfloat16, kind="ExternalInput")
    # ct[p, k, b] = (b_f + hidden @ W_h + b_h)[b, k*128 + p]  (f32)
    ct = nc.dram_tensor("ct", [128, IC, BSH], dt.float32, kind="ExternalInput")
    # ba[p, c] = b_a[c*128 + p]  (f32)
    ba = nc.dram_tensor("ba", [128, DC], dt.float32, kind="ExternalInput")
    # outputs
    alphat = nc.dram_tensor(
        "alphat", [BSH, 128, DC, L], dt.bfloat16, kind="ExternalOutput"
    )
    zt = nc.dram_tensor("zt", [BSH, 128, DC], dt.float32, kind="ExternalOutput")

    with tile.TileContext(nc) as tc:
        with (
            tc.tile_pool(name="const", bufs=1) as cpool,
            tc.tile_pool(name="xin", bufs=2) as xpool,
            tc.tile_pool(name="t1", bufs=2) as t1pool,
            tc.tile_pool(name="expe", bufs=2) as epool,
            tc.tile_pool(name="alpha", bufs=4) as apool,
            tc.tile_pool(name="stats", bufs=2) as spool,
            tc.tile_pool(name="scratch", bufs=2) as scpool,
            tc.tile_pool(name="ps1", bufs=4, space="PSUM") as ps1,
            tc.tile_pool(name="ps2", bufs=4, space="PSUM") as ps2,
        ):
            # ---- load constants / weights once ----
            wf_sb = cpool.tile([128, DC, I], dt.bfloat16)
            nc.sync.dma_start(wf_sb[:], wf.ap())
            wa_sb = cpool.tile([128, IC, D], dt.bfloat16)
            nc.sync.dma_start(wa_sb[:], wa.ap())
            ct_sb = cpool.tile([128, IC, BSH], dt.float32)
            nc.sync.dma_start(ct_sb[:], ct.ap())
            ba_sb = cpool.tile([128, DC], dt.float32)
            nc.sync.dma_start(ba_sb[:], ba.ap())

            for pr in range(NPAIR):
                b0 = 2 * pr
                # xt_sb[p, h, c, l], bf16 (cast during SWDGE DMA)
                xt_sb = xpool.tile([128, 2, DC, L], dt.bfloat16, tag="xt")
                nc.gpsimd.dma_start(xt_sb[:, 0], xt.ap()[b0])
                nc.gpsimd.dma_start(xt_sb[:, 1], xt.ap()[b0 + 1])

                # ---- matmul 1: t1T[i, (h,l)] = sum_d W_f[d,i] * xT[d,(h,l)]
                t1s = t1pool.tile([128, IC, 2, L], dt.bfloat16, tag="t1s")
                for m in range(IC):
                    pt1 = ps1.tile([128, 2, L], dt.float32, tag="pt1")
                    for k in range(DC):
                        nc.tensor.matmul(
                            pt1[:],
                            wf_sb[:, k, m * 128 : (m + 1) * 128],
                            xt_sb[:, :, k, :],
                            start=(k == 0),
                            stop=(k == DC - 1),
                        )
                    # bias add (per-partition, per-batch) + cast to bf16
                    for h in range(2):
                        nc.scalar.activation(
                            t1s[:, m, h, :],
                            pt1[:, h, :],
                            AF.Identity,
                            bias=ct_sb[:, m, b0 + h : b0 + h + 1],
                            scale=1.0,
                        )

                # ---- matmul 2 + exp + row-sum ----
                expe = epool.tile([128, DC, 2, L], dt.bfloat16, tag="expe")
                S = spool.tile([128, DC, 2], dt.float32, tag="S")
                for mc in range(DC):
                    pe = ps2.tile([128, 2, L], dt.float32, tag="pe")
                    for k in range(IC):
                        nc.tensor.matmul(
                            pe[:],
                            wa_sb[:, k, mc * 128 : (mc + 1) * 128],
                            t1s[:, k, :, :],
                            start=(k == 0),
                            stop=(k == IC - 1),
                        )
                    for h in range(2):
                        nc.scalar.activation(
                            expe[:, mc, h, :],
                            pe[:, h, :],
                            AF.Exp,
                            bias=ba_sb[:, mc : mc + 1],
                            scale=1.0,
                            accum_out=S[:, mc, h : h + 1],
                        )

                # ---- softmax denominators ----
                recip = spool.tile([128, DC, 2], dt.float32, tag="recip")
                nc.vector.reciprocal(recip[:], S[:])

                # ---- normalize, weighted sum, store ----
                for h in range(2):
                    ast = apool.tile([128, DC, L], dt.bfloat16, tag="ast")
                    zsb = spool.tile([128, DC], dt.float32, tag="zsb")
                    for mc in range(DC):
                        nc.vector.tensor_scalar_mul(
                            ast[:, mc, :],
                            expe[:, mc, h, :],
                            recip[:, mc, h : h + 1],
                        )
                        scr = scpool.tile([128, L], dt.bfloat16, tag="scr")
                        # out = (expe * recip) * xT ; accum_out = z chunk
                        nc.vector.scalar_tensor_tensor(
                            out=scr[:],
                            in0=expe[:, mc, h, :],
                            scalar=recip[:, mc, h : h + 1],
                            in1=xt_sb[:, h, mc, :],
                            op0=ALU.mult,
                            op1=ALU.mult,
                            accum_out=zsb[:, mc : mc + 1],
                        )
                    nc.sync.dma_start(alphat.ap()[b0 + h], ast[:])
                    nc.sync.dma_start(zt.ap()[b0 + h], zsb[:])

    nc.compile()
    return nc


def _get_nc():
    if "nc" not in _CACHE:
        _CACHE["nc"] = _build_nc()
    return _CACHE["nc"]


def kernel(feature_vectors, hidden_state, W_f, b_f, W_h, b_h, W_a, b_a):
    import ml_dtypes

    from concourse.bass_utils import run_bass_kernel_spmd

    bf16 = ml_dtypes.bfloat16

    fv = np.asarray(feature_vectors, dtype=np.float32)
    hidden = np.asarray(hidden_state, dtype=np.float32)
    W_f = np.asarray(W_f, dtype=np.float32)
    b_f = np.asarray(b_f, dtype=np.float32)
    W_h = np.asarray(W_h, dtype=np.float32)
    b_h = np.asarray(b_h, dtype=np.float32)
    W_a = np.asarray(W_a, dtype=np.float32)
    b_a = np.asarray(b_a, dtype=np.float32)

    # ---- host-side preprocessing (cheap, part of sharding) ----
    # combined per-batch bias: c[b, i] = b_f[i] + (hidden @ W_h + b_h)[b, i]
    c_all = b_f[None, :] + hidden @ W_h + b_h[None, :]          # [B, I] f32

    wf_dev = np.ascontiguousarray(
        W_f.reshape(DC, 128, I).transpose(1, 0, 2)
    ).astype(bf16)                                              # [128, DC, I]
    wa_dev = np.ascontiguousarray(
        W_a.reshape(IC, 128, D).transpose(1, 0, 2)
    ).astype(bf16)                                              # [128, IC, D]
    ba_dev = np.ascontiguousarray(b_a.reshape(DC, 128).T)       # [128, DC]

    # xt[b, p, c, l] = fv[b, l, c*128+p]
    xt_all = np.ascontiguousarray(
        fv.reshape(B, L, DC, 128).transpose(0, 3, 2, 1)
    )                                                           # [B, 128, DC, L]

    in_maps = []
    for core in range(NCORES):
        sl = slice(core * BSH, (core + 1) * BSH)
        c_sh = c_all[sl]                                        # [BSH, I]
        ct_dev = np.ascontiguousarray(
            c_sh.T.reshape(IC, 128, BSH).transpose(1, 0, 2)
        )                                                       # [128, IC, BSH]
        in_maps.append(
            {
                "xt": np.ascontiguousarray(xt_all[sl]),
                "wf": wf_dev,
                "wa": wa_dev,
                "ct": ct_dev,
                "ba": ba_dev,
            }
        )

    nc = _get_nc()
    res = run_bass_kernel_spmd(nc, in_maps, core_ids=list(range(NCORES)))

    # ---- gather / unshard ----
    alphat = np.concatenate(
        [np.asarray(r["alphat"]) for r in res.results], axis=0
    )                                                           # [B,128,DC,L] bf16
    ztv = np.concatenate([np.asarray(r["zt"]) for r in res.results], axis=0)

    alpha = np.ascontiguousarray(
        alphat.astype(np.float32).transpose(0, 3, 2, 1)
    ).reshape(B, L, D)
    z = np.ascontiguousarray(ztv.transpose(0, 2, 1)).reshape(B, D)
    return z, alpha


# revision 3
# speedup vs baseline: 1.1006x; 1.1006x over previous
"""Trainium2 Bass kernel for nn_Attention_78847009620267.

Reference computation (per batch b):
    t1 = fv[b] @ W_f + (b_f + hidden[b] @ W_h + b_h)     # [L, I]
    e  = t1 @ W_a + b_a                                  # [L, D]
    alpha = softmax(e, axis=L)                           # [L, D]
    z  = sum_l alpha * fv[b]                             # [D]
returns (z [B, D], alpha [B, L, D])

Strategy: data-parallel over batch B=64 across 8 NeuronCores (8 batches per
core), weights replicated.  The whole device kernel runs in the transposed
[D, L] domain so that softmax reductions, bias adds, and normalization are
all native per-partition operations; the host pre-transposes fv into
[b, p, c, l] layout (d = c*128 + p) and post-transposes alpha back.  All
matmuls run in bf16 (fp32 accumulation in PSUM); exp's softmax denominator
comes for free from the ScalarE activation accum_out.

No collectives are needed: each core's outputs depend only on its shard.
"""

import os
import sys

import numpy as np

sys.path.insert(0, "/opt/trn_rl_repo")

B, L, D, I = 64, 196, 2048, 512
NCORES = 8
BSH = B // NCORES          # batches per core = 8
DC = D // 128              # 16 d-chunks
IC = I // 128              # 4 i-chunks
NPAIR = BSH // 2           # process 2 batches per pass (N = 2*L = 392)
SGRP = 4                   # alpha store granularity (d-chunks per store)

_CACHE = {}


def _build_nc():
    """Build the per-core Bass graph (same graph for all 8 cores)."""
    import concourse.bass as bass  # noqa: F401
    import concourse.mybir as mybir
    import concourse.tile as tile
    from concourse import bacc

    dt = mybir.dt
    AF = mybir.ActivationFunctionType
    ALU = mybir.AluOpType

    nc = bacc.Bacc("TRN2", target_bir_lowering=False, debug=False)

    # ---- I/O declarations (per-core shard layouts, see host code below) ----
    # xt[b, p, c, l] = fv[b, l, c*128 + p]  (f32; cast to bf16 during DMA)
    xt = nc.dram_tensor("xt", [BSH, 128, DC, L], dt.float32, kind="ExternalInput")
    # wf[p, k, i] = W_f[k*128 + p, i]  (bf16)
    wf = nc.dram_tensor("wf", [128, DC, I], dt.bfloat16, kind="ExternalInput")
    # wa[p, k, d] = W_a[k*128 + p, d]  (bf16)
    wa = nc.dram_tensor("wa", [128, IC, D], dt.bfloat16, kind="ExternalInput")
    # ct[p, k, b] = (b_f + hidden @ W_h + b_h)[b, k*128 + p]  (f32)
    ct = nc.dram_tensor("ct", [128, IC, BSH], dt.float32, kind="ExternalInput")
    # ba[p, c] = b_a[c*128 + p]  (f32)
    ba = nc.dram_tensor("ba", [128, DC], dt.float32, kind="ExternalInput")
    # outputs
    alphat = nc.dram_tensor(
        "alphat", [BSH, 128, DC, L], dt.bfloat16, kind="ExternalOutput"
    )
    zt = nc.dram_tensor("zt", [BSH, 128, DC], dt.float32, kind="ExternalOutput")

    with tile.TileContext(nc) as tc:
        with (
            tc.tile_pool(name="const", bufs=1) as cpool,
            tc.tile_pool(name="xin", bufs=3) as xpool,
            tc.tile_pool(name="t1", bufs=2) as t1pool,
            tc.tile_pool(name="expe", bufs=2) as epool,
            tc.tile_pool(name="alpha", bufs=4) as apool,
            tc.tile_pool(name="stats", bufs=3) as spool,
            tc.tile_pool(name="scratch", bufs=2) as scpool,
            tc.tile_pool(name="ps1", bufs=4, space="PSUM") as ps1,
            tc.tile_pool(name="ps2", bufs=4, space="PSUM") as ps2,
        ):
            # ---- first xt tile before weights, so mm1 can start early ----
            xts = []
            xt0 = xpool.tile([128, 2, DC, L], dt.bfloat16, tag="xt", name="xt0")
            nc.gpsimd.dma_start(xt0[:, 0], xt.ap()[0])
            nc.gpsimd.dma_start(xt0[:, 1], xt.ap()[1])
            xts.append(xt0)

            # ---- load constants / weights (wf split so k=0 arrives early) ----
            wf_sb = cpool.tile([128, DC, I], dt.bfloat16)
            for kg in range(4):
                nc.sync.dma_start(
                    wf_sb[:, 4 * kg : 4 * (kg + 1), :],
                    wf.ap()[:, 4 * kg : 4 * (kg + 1), :],
                )
            ct_sb = cpool.tile([128, IC, BSH], dt.float32)
            nc.sync.dma_start(ct_sb[:], ct.ap())
            ba_sb = cpool.tile([128, DC], dt.float32)
            nc.sync.dma_start(ba_sb[:], ba.ap())
            wa_sb = cpool.tile([128, IC, D], dt.bfloat16)
            for kg in range(2):
                nc.sync.dma_start(
                    wa_sb[:, 2 * kg : 2 * (kg + 1), :],
                    wa.ap()[:, 2 * kg : 2 * (kg + 1), :],
                )

            for pr in range(NPAIR):
                b0 = 2 * pr
                xt_sb = xts[pr]
                # prefetch next pair's xt
                if pr + 1 < NPAIR:
                    xt_n = xpool.tile(
                        [128, 2, DC, L], dt.bfloat16, tag="xt", name=f"xt{pr + 1}"
                    )
                    nc.gpsimd.dma_start(xt_n[:, 0], xt.ap()[b0 + 2])
                    nc.gpsimd.dma_start(xt_n[:, 1], xt.ap()[b0 + 3])
                    xts.append(xt_n)

                # ---- matmul 1: t1T[i, (h,l)] = sum_d W_f[d,i] * xT[d,(h,l)]
                t1s = t1pool.tile([128, IC, 2, L], dt.bfloat16, tag="t1s")
                for m in range(IC):
                    pt1 = ps1.tile([128, 2, L], dt.float32, tag="pt1")
                    for k in range(DC):
                        nc.tensor.matmul(
                            pt1[:],
                            wf_sb[:, k, m * 128 : (m + 1) * 128],
                            xt_sb[:, :, k, :],
                            start=(k == 0),
                            stop=(k == DC - 1),
                        )
                    # bias add (per-partition, per-batch) + cast to bf16 (DVE)
                    for h in range(2):
                        nc.vector.tensor_scalar_add(
                            t1s[:, m, h, :],
                            pt1[:, h, :],
                            ct_sb[:, m, b0 + h : b0 + h + 1],
                        )

                # ---- matmul 2 + exp + streamed softmax/normalize/z ----
                S = spool.tile([128, DC, 2], dt.float32, tag="S")
                recip = spool.tile([128, DC, 2], dt.float32, tag="recip")
                zraw = spool.tile([128, 2, DC], dt.float32, tag="zraw")
                asts = [
                    apool.tile([128, DC, L], dt.bfloat16, tag="ast", name=f"ast{pr}_{h}")
                    for h in range(2)
                ]
                for mc in range(DC):
                    pe = ps2.tile([128, 2, L], dt.float32, tag="pe")
                    for k in range(IC):
                        nc.tensor.matmul(
                            pe[:],
                            wa_sb[:, k, mc * 128 : (mc + 1) * 128],
                            t1s[:, k, :, :],
                            start=(k == 0),
                            stop=(k == IC - 1),
                        )
                    expe = scpool.tile([128, 2, L], dt.bfloat16, tag="expe")
                    for h in range(2):
                        nc.scalar.activation(
                            expe[:, h, :],
                            pe[:, h, :],
                            AF.Exp,
                            bias=ba_sb[:, mc : mc + 1],
                            scale=1.0,
                            accum_out=S[:, mc, h : h + 1],
                        )
                    # z contribution from unnormalized exp (no recip dep)
                    scr = scpool.tile([128, L], dt.bfloat16, tag="scr")
                    for h in range(2):
                        nc.vector.scalar_tensor_tensor(
                            out=scr[:],
                            in0=expe[:, h, :],
                            scalar=1.0,
                            in1=xt_sb[:, h, mc, :],
                            op0=ALU.mult,
                            op1=ALU.mult,
                            accum_out=zraw[:, h, mc : mc + 1],
                        )
                    # per-chunk softmax denominator + normalize
                    nc.vector.reciprocal(recip[:, mc, :], S[:, mc, :])
                    for h in range(2):
                        nc.vector.tensor_scalar_mul(
                            asts[h][:, mc, :],
                            expe[:, h, :],
                            recip[:, mc, h : h + 1],
                        )
                    # store alpha in groups as chunks complete
                    if (mc + 1) % SGRP == 0:
                        g0 = mc + 1 - SGRP
                        for h in range(2):
                            nc.sync.dma_start(
                                alphat.ap()[b0 + h, :, g0 : mc + 1, :],
                                asts[h][:, g0 : mc + 1, :],
                            )

                # ---- z = zraw * recip  (tiny per-partition scale) ----
                zsb = spool.tile([128, 2, DC], dt.float32, tag="zsb")
                for h in range(2):
                    nc.vector.tensor_tensor(
                        zsb[:, h, :],
                        zraw[:, h, :],
                        recip[:, :, h],
                        ALU.mult,
                    )
                    nc.sync.dma_start(zt.ap()[b0 + h], zsb[:, h, :])

    nc.compile()
    return nc


def _get_nc():
    if "nc" not in _CACHE:
        _CACHE["nc"] = _build_nc()
    return _CACHE["nc"]


def _host_prep(fv, hidden, W_f, b_f, W_h, b_h, W_a, b_a):
    import ml_dtypes

    bf16 = ml_dtypes.bfloat16
    c_all = b_f[None, :] + hidden @ W_h + b_h[None, :]          # [B, I] f32

    wf_dev = np.ascontiguousarray(
        W_f.reshape(DC, 128, I).transpose(1, 0, 2)
    ).astype(bf16)                                              # [128, DC, I]
    wa_dev = np.ascontiguousarray(
        W_a.reshape(IC, 128, D).transpose(1, 0, 2)
    ).astype(bf16)                                              # [128, IC, D]
    ba_dev = np.ascontiguousarray(b_a.reshape(DC, 128).T)       # [128, DC]

    # xt[b, p, c, l] = fv[b, l, c*128+p]
    xt_all = np.ascontiguousarray(
        fv.reshape(B, L, DC, 128).transpose(0, 3, 2, 1)
    )                                                           # [B, 128, DC, L]

    in_maps = []
    for core in range(NCORES):
        sl = slice(core * BSH, (core + 1) * BSH)
        ct_dev = np.ascontiguousarray(
            c_all[sl].T.reshape(IC, 128, BSH).transpose(1, 0, 2)
        )                                                       # [128, IC, BSH]
        in_maps.append(
            {
                "xt": np.ascontiguousarray(xt_all[sl]),
                "wf": wf_dev,
                "wa": wa_dev,
                "ct": ct_dev,
                "ba": ba_dev,
            }
        )
    return in_maps


def kernel(feature_vectors, hidden_state, W_f, b_f, W_h, b_h, W_a, b_a):
    from concourse.bass_utils import run_bass_kernel_spmd

    fv = np.asarray(feature_vectors, dtype=np.float32)
    hidden = np.asarray(hidden_state, dtype=np.float32)
    in_maps = _host_prep(
        fv,
        hidden,
        np.asarray(W_f, dtype=np.float32),
        np.asarray(b_f, dtype=np.float32),
        np.asarray(W_h, dtype=np.float32),
        np.asarray(b_h, dtype=np.float32),
        np.asarray(W_a, dtype=np.float32),
        np.asarray(b_a, dtype=np.float32),
    )

    nc = _get_nc()
    res = run_bass_kernel_spmd(nc, in_maps, core_ids=list(range(NCORES)))

    # ---- gather / unshard ----
    alphat = np.concatenate(
        [np.asarray(r["alphat"]) for r in res.results], axis=0
    )                                                           # [B,128,DC,L] bf16
    ztv = np.concatenate([np.asarray(r["zt"]) for r in res.results], axis=0)

    alpha = np.ascontiguousarray(
        alphat.astype(np.float32).transpose(0, 3, 2, 1)
    ).reshape(B, L, D)
    z = np.ascontiguousarray(ztv.transpose(0, 2, 1)).reshape(B, D)
    return z, alpha


# revision 7
# speedup vs baseline: 1.2946x; 1.1762x over previous
"""Trainium2 Bass kernel for nn_Attention_78847009620267.

Reference computation (per batch b):
    t1 = fv[b] @ W_f + (b_f + hidden[b] @ W_h + b_h)     # [L, I]
    e  = t1 @ W_a + b_a                                  # [L, D]
    alpha = softmax(e, axis=L)                           # [L, D]
    z  = sum_l alpha * fv[b]                             # [D]
returns (z [B, D], alpha [B, L, D])

Strategy: data-parallel over batch B=64 across 8 NeuronCores (8 batches per
core), weights replicated.  The whole device kernel runs in the transposed
[D, L] domain so that softmax reductions, bias adds, and normalization are
all native per-partition operations; the host pre-transposes fv into
[b, p, c, l] bf16 layout (d = c*128 + p) and post-transposes alpha back.
All matmuls run in bf16 (fp32 accumulation in PSUM); exp's softmax
denominator comes for free from the ScalarE activation accum_out.

No collectives are needed: each core's outputs depend only on its shard.
"""

import os
import sys

import numpy as np

sys.path.insert(0, "/opt/trn_rl_repo")

B, L, D, I = 64, 196, 2048, 512
NCORES = 8
BSH = B // NCORES          # batches per core = 8
DC = D // 128              # 16 d-chunks
IC = I // 128              # 4 i-chunks
NPAIR = BSH // 2           # process 2 batches per pass (N = 2*L = 392)
XG = 2                     # xt tiles per pair (chunk groups)
GCH = DC // XG             # chunks per xt group

_CACHE = {}


def _build_nc():
    """Build the per-core Bass graph (same graph for all 8 cores)."""
    import concourse.bass as bass  # noqa: F401
    import concourse.mybir as mybir
    import concourse.tile as tile
    from concourse import bacc

    dt = mybir.dt
    AF = mybir.ActivationFunctionType
    ALU = mybir.AluOpType

    nc = bacc.Bacc("TRN2", target_bir_lowering=False, debug=False)

    # ---- I/O declarations (per-core shard layouts, see host code below) ----
    # xt[b, p, c, l] = bf16(fv[b, l, c*128 + p])
    xt = nc.dram_tensor("xt", [BSH, 128, DC, L], dt.bfloat16, kind="ExternalInput")
    # wf[p, k, i] = W_f[k*128 + p, i]  (bf16)
    wf = nc.dram_tensor("wf", [128, DC, I], dt.bfloat16, kind="ExternalInput")
    # wa[p, k, d] = W_a[k*128 + p, d]  (bf16)
    wa = nc.dram_tensor("wa", [128, IC, D], dt.bfloat16, kind="ExternalInput")
    # ct[p, k, b] = (b_f + hidden @ W_h + b_h)[b, k*128 + p]  (f32)
    ct = nc.dram_tensor("ct", [128, IC, BSH], dt.float32, kind="ExternalInput")
    # ba[p, c] = b_a[c*128 + p]  (f32)
    ba = nc.dram_tensor("ba", [128, DC], dt.float32, kind="ExternalInput")
    # outputs
    alphat = nc.dram_tensor(
        "alphat", [BSH, 128, DC, L], dt.bfloat16, kind="ExternalOutput"
    )
    zt = nc.dram_tensor("zt", [BSH, 128, DC], dt.float32, kind="ExternalOutput")

    with tile.TileContext(nc) as tc:
        with (
            tc.tile_pool(name="const", bufs=1) as cpool,
            tc.tile_pool(name="xin", bufs=2 * XG + 2) as xpool,
            tc.tile_pool(name="t1", bufs=2) as t1pool,
            tc.tile_pool(name="alpha", bufs=4) as apool,
            tc.tile_pool(name="stats", bufs=3) as spool,
            tc.tile_pool(name="scratch", bufs=3) as scpool,
            tc.tile_pool(name="ps1", bufs=4, space="PSUM") as ps1,
            tc.tile_pool(name="ps2", bufs=4, space="PSUM") as ps2,
        ):
            # xt loads ride the Sync HWDGE queue (with alpha stores);
            # weights ride the Scalar HWDGE queue so they don't contend.
            def load_pair(pr):
                b0 = 2 * pr
                tiles = []
                for g in range(XG):
                    t = xpool.tile(
                        [128, 2, GCH, L], dt.bfloat16, tag="xtg", name=f"xt{pr}_{g}"
                    )
                    for h in range(2):
                        nc.sync.dma_start(
                            t[:, h],
                            xt.ap()[b0 + h, :, g * GCH : (g + 1) * GCH, :],
                        )
                    tiles.append(t)
                return tiles

            xts = {0: load_pair(0)}

            wf_g = []
            for kg in range(4):
                t = cpool.tile([128, 4, I], dt.bfloat16, name=f"wf{kg}")
                nc.scalar.dma_start(t[:], wf.ap()[:, 4 * kg : 4 * (kg + 1), :])
                wf_g.append(t)
            ct_sb = cpool.tile([128, IC, BSH], dt.float32)
            nc.scalar.dma_start(ct_sb[:], ct.ap())
            ba_sb = cpool.tile([128, DC], dt.float32)
            nc.scalar.dma_start(ba_sb[:], ba.ap())
            wa_g = []
            for kg in range(2):
                t = cpool.tile([128, 2, D], dt.bfloat16, name=f"wa{kg}")
                nc.scalar.dma_start(t[:], wa.ap()[:, 2 * kg : 2 * (kg + 1), :])
                wa_g.append(t)

            for pr in range(NPAIR):
                b0 = 2 * pr
                xt_sb = xts[pr]

                # ---- matmul 1: t1T[i, (h,l)] = sum_d W_f[d,i] * xT[d,(h,l)]
                t1s = t1pool.tile([128, IC, 2, L], dt.bfloat16, tag="t1s")
                for m in range(IC):
                    pt1 = ps1.tile([128, 2, L], dt.float32, tag="pt1")
                    for k in range(DC):
                        nc.tensor.matmul(
                            pt1[:],
                            wf_g[k // 4][:, k % 4, m * 128 : (m + 1) * 128],
                            xt_sb[k // GCH][:, :, k % GCH, :],
                            start=(k == 0),
                            stop=(k == DC - 1),
                        )
                    # bias add (per-partition, per-batch) + cast to bf16 (DVE)
                    for h in range(2):
                        nc.vector.tensor_scalar_add(
                            t1s[:, m, h, :],
                            pt1[:, h, :],
                            ct_sb[:, m, b0 + h : b0 + h + 1],
                        )

                # prefetch next pair's xt (after mm1 so it doesn't gate it)
                if pr + 1 < NPAIR:
                    xts[pr + 1] = load_pair(pr + 1)

                # ---- matmul 2 + exp + streamed softmax/normalize/z ----
                sgrp = 4 if pr + 1 < NPAIR else 2
                S = spool.tile([128, DC, 2], dt.float32, tag="S")
                recip = spool.tile([128, DC, 2], dt.float32, tag="recip")
                zraw = spool.tile([128, 2, DC], dt.float32, tag="zraw")
                asts = [
                    apool.tile([128, DC, L], dt.bfloat16, tag="ast", name=f"ast{pr}_{h}")
                    for h in range(2)
                ]
                expes = {}
                for mc in range(DC):
                    pe = ps2.tile([128, 2, L], dt.float32, tag="pe")
                    for k in range(IC):
                        nc.tensor.matmul(
                            pe[:],
                            wa_g[k // 2][:, k % 2, mc * 128 : (mc + 1) * 128],
                            t1s[:, k, :, :],
                            start=(k == 0),
                            stop=(k == IC - 1),
                        )
                    expe = scpool.tile([128, 2, L], dt.bfloat16, tag="expe")
                    expes[mc] = expe
                    for h in range(2):
                        nc.scalar.activation(
                            expe[:, h, :],
                            pe[:, h, :],
                            AF.Exp,
                            bias=ba_sb[:, mc : mc + 1],
                            scale=1.0,
                            accum_out=S[:, mc, h : h + 1],
                        )
                    # z contribution from unnormalized exp (no recip dep)
                    for h, eng in ((0, nc.vector), (1, nc.vector)):
                        scr = scpool.tile([128, L], dt.bfloat16, tag=f"scr{h}")
                        eng.scalar_tensor_tensor(
                            out=scr[:],
                            in0=expe[:, h, :],
                            scalar=1.0,
                            in1=xt_sb[mc // GCH][:, h, mc % GCH, :],
                            op0=ALU.mult,
                            op1=ALU.mult,
                            accum_out=zraw[:, h, mc : mc + 1],
                        )
                    # softmax denominators every 2 chunks; normalize
                    if mc % 2 == 1:
                        nc.vector.reciprocal(
                            recip[:, mc - 1 : mc + 1, :], S[:, mc - 1 : mc + 1, :]
                        )
                        for mcc in (mc - 1, mc):
                            for h in range(2):
                                nc.vector.tensor_scalar_mul(
                                    asts[h][:, mcc, :],
                                    expes[mcc][:, h, :],
                                    recip[:, mcc, h : h + 1],
                                )
                    # store alpha in groups as chunks complete
                    if (mc + 1) % sgrp == 0:
                        g0 = mc + 1 - sgrp
                        for h in range(2):
                            nc.sync.dma_start(
                                alphat.ap()[b0 + h, :, g0 : mc + 1, :],
                                asts[h][:, g0 : mc + 1, :],
                            )

                # ---- z = zraw * recip  (tiny per-partition scale) ----
                zsb = spool.tile([128, 2, DC], dt.float32, tag="zsb")
                for h in range(2):
                    nc.vector.tensor_tensor(
                        zsb[:, h, :],
                        zraw[:, h, :],
                        recip[:, :, h],
                        ALU.mult,
                    )
                    nc.sync.dma_start(zt.ap()[b0 + h], zsb[:, h, :])

    nc.compile()
    return nc


def _get_nc():
    if "nc" not in _CACHE:
        _CACHE["nc"] = _build_nc()
    return _CACHE["nc"]


def _host_prep(fv, hidden, W_f, b_f, W_h, b_h, W_a, b_a):
    import ml_dtypes

    bf16 = ml_dtypes.bfloat16
    c_all = b_f[None, :] + hidden @ W_h + b_h[None, :]          # [B, I] f32

    wf_dev = np.ascontiguousarray(
        W_f.reshape(DC, 128, I).transpose(1, 0, 2)
    ).astype(bf16)                                              # [128, DC, I]
    wa_dev = np.ascontiguousarray(
        W_a.reshape(IC, 128, D).transpose(1, 0, 2)
    ).astype(bf16)                                              # [128, IC, D]
    ba_dev = np.ascontiguousarray(b_a.reshape(DC, 128).T)       # [128, DC]

    # xt[b, p, c, l] = bf16(fv[b, l, c*128+p])
    xt_all = np.ascontiguousarray(
        fv.reshape(B, L, DC, 128).transpose(0, 3, 2, 1)
    ).astype(bf16)                                              # [B, 128, DC, L]

    in_maps = []
    for core in range(NCORES):
        sl = slice(core * BSH, (core + 1) * BSH)
        ct_dev = np.ascontiguousarray(
            c_all[sl].T.reshape(IC, 128, BSH).transpose(1, 0, 2)
        )                                                       # [128, IC, BSH]
        in_maps.append(
            {
                "xt": np.ascontiguousarray(xt_all[sl]),
                "wf": wf_dev,
                "wa": wa_dev,
                "ct": ct_dev,
                "ba": ba_dev,
            }
        )
    return in_maps


def kernel(feature_vectors, hidden_state, W_f, b_f, W_h, b_h, W_a, b_a):
    from concourse.bass_utils import run_bass_kernel_spmd

    fv = np.asarray(feature_vectors, dtype=np.float32)
    hidden = np.asarray(hidden_state, dtype=np.float32)
    in_maps = _host_prep(
        fv,
        hidden,
        np.asarray(W_f, dtype=np.float32),
        np.asarray(b_f, dtype=np.float32),
        np.asarray(W_h, dtype=np.float32),
        np.asarray(b_h, dtype=np.float32),
        np.asarray(W_a, dtype=np.float32),
        np.asarray(b_a, dtype=np.float32),
    )

    nc = _get_nc()
    res = run_bass_kernel_spmd(nc, in_maps, core_ids=list(range(NCORES)))

    # ---- gather / unshard ----
    alphat = np.concatenate(
        [np.asarray(r["alphat"]) for r in res.results], axis=0
    )                                                           # [B,128,DC,L] bf16
    ztv = np.concatenate([np.asarray(r["zt"]) for r in res.results], axis=0)

    alpha = np.ascontiguousarray(
        alphat.astype(np.float32).transpose(0, 3, 2, 1)
    ).reshape(B, L, D)
    z = np.ascontiguousarray(ztv.transpose(0, 2, 1)).reshape(B, D)
    return z, alpha
